# revision 1
# baseline (speedup 1.0000x reference)
"""Trainium2 Bass kernel for nn_DTFormer (histogram_binning).

Math: for each batch row and each of src/dst lists, count (id,snap)
multiset matches (self and cross), then run the counts through two tiny
MLPs.  Since the MLP output depends only on (self_count, cross_count,
snap) -- integers with tiny range -- the whole MLP pipeline is
precomputed host-side into a lookup table T[32*32*8, 128] from the
params.  The device kernel does the real work: the match counting and a
row-gather of T, data-parallel over the 64 batch rows across 8 cores.

Inputs are packed host-side into a single combined key per element:
v = 8*id + (snap-1) < 16384; equality of v <=> equality of (id, snap).
v is supplied both as int16 (for 2x-mode DVE compares) and f32 (for
per-partition scalar operands); valid = (v >= 8).

Counting layout per batch row: E_xy[j, k] = [v_x[j] == v_y[k]] is
processed in 8 j-tiles of [128 x 1024].  Row sums come free via the
compare op's accum_out; dst-cross counts are column sums of E_sd,
accumulated on the PE with a ones-matmul and round-tripped through DRAM
to transpose the layout.
"""

import sys

for p in ("/opt/trn_rl_repo", "/root/.axon_site/_ro/trn_rl_repo"):
    if p not in sys.path:
        sys.path.insert(0, p)

import numpy as np
from contextlib import ExitStack

import concourse.bass as bass
import concourse.bacc as bacc
import concourse.tile as tile
from concourse import mybir
from concourse.bass_utils import run_bass_kernel_spmd

B, L, S, D = 64, 1024, 8, 128
NCORES = 8
BPC = B // NCORES          # batches per core
NT = L // 128              # j-tiles per row
CMAX = 32                  # count clamp (counts are tiny; 32 is ample)
TROWS = CMAX * CMAX * S    # 8192 table rows
N_ACT_DD = 7               # dd j-tiles handled by PE+ACT (rest on DVE)

F32 = mybir.dt.float32
F16 = mybir.dt.float16
BF16 = mybir.dt.bfloat16
I16 = mybir.dt.int16
I32 = mybir.dt.int32
ALU = mybir.AluOpType
ACTF = mybir.ActivationFunctionType

_NC_CACHE = {}
DEBUG_COUNTS = False
TRACE = False
LAST_RESULTS = {}


def build_table(agg_w1, agg_b1, agg_w2, agg_b2, enc_w1, enc_b1, enc_w2, enc_b2):
    """T[a*CMAX*S + b*S + s] = output row for (self=a, cross=b, snap=s+1)."""
    a = np.arange(CMAX, dtype=np.float64)
    w1 = agg_w1.astype(np.float64)      # [S, D]
    b1 = agg_b1.astype(np.float64)      # [D]
    ha = np.maximum(a[None, :, None] * w1[:, None, :] + b1, 0.0)  # [S, CMAX, D]
    g = 0.5 * (ha[:, :, None, :] + ha[:, None, :, :])             # [S, A, B, D]
    y = g @ agg_w2.astype(np.float64) + agg_b2.astype(np.float64)  # [S, A, B, 2]
    ew1 = enc_w1.astype(np.float64)[0]   # [D]
    eb1 = enc_b1.astype(np.float64)
    h0 = np.maximum(y[..., 0:1] * ew1 + eb1, 0.0)  # [S, A, B, D]
    h1 = np.maximum(y[..., 1:2] * ew1 + eb1, 0.0)
    out = (h0 + h1) @ enc_w2.astype(np.float64) + 2.0 * enc_b2.astype(np.float64)
    out = np.transpose(out, (1, 2, 0, 3)).reshape(TROWS, D)  # [A,B,S,D] flat
    return np.ascontiguousarray(out.astype(np.float32))


def _replicate_ap(row_ap, parts=128):
    """AP that reads a DRAM row [N] replicated across `parts` partitions."""
    return bass.AP(tensor=row_ap.tensor, offset=row_ap.offset,
                   ap=[[0, parts]] + [list(p) for p in row_ap.ap])


def _flush_gathers(nc, tc, drsc, feat, table, feat_t, pend):
    # round-trip keys through DRAM into the wrapped/replicated int16 idx
    # layout dma_gather expects: idxs[16g + i, w] = key[16w + i] for every g.
    # Keys of several lists are concatenated so the 8 wrap DMAs are shared.
    n = len(pend)
    ctx_hp = tc.high_priority()
    ctx_hp.__enter__()
    # keys are written to DRAM in the raw [p, t] tile order (contiguous),
    # and wrap stripes are read contiguously; the induced permutation of
    # gather slots is undone by the store AP: slot (p, q) holds output row
    # j = 8p + q.
    k_scr = drsc.tile([1, n * L], I16, tag="kscr", name="k_scr")
    for q, (b_, x_, kt) in enumerate(pend):
        nc.sync.dma_start(
            out=k_scr[0, q * L:(q + 1) * L].rearrange("(p t) -> p t", t=NT),
            in_=kt[:])
    idxs_sb = feat.tile([128, n, L // 16], I16, tag="idxs", name="idxs_sb")
    wrap_ap = k_scr[0, :].rearrange("(q i w) -> i q w", i=16, w=L // 16)
    for g in range(8):
        nc.sync.dma_start(out=idxs_sb[16 * g:16 * (g + 1), :, :], in_=wrap_ap)
    for q, (b_, x_, kt) in enumerate(pend):
        ft = feat.tile([128, NT, D], F32, tag="ft", name="ft")
        nc.gpsimd.dma_gather(
            out_ap=ft[:], in_ap=table[:],
            idxs_ap=idxs_sb[:, q, :],
            num_idxs=L, num_idxs_reg=L, elem_size=D)
        nc.gpsimd.dma_start(
            out=feat_t[x_][b_, :, :].rearrange("(p q) d -> p q d", q=NT),
            in_=ft[:])
    ctx_hp.__exit__(None, None, None)
    pend.clear()


def build_nc():
    nc = bacc.Bacc("TRN2")
    vsn_d = nc.dram_tensor("vsn", [BPC, 128, 2, 2, NT], F32, kind="ExternalInput")
    # K=8 fp16 operands for the PE distance matmul (dst list), all values
    # integer-exact in fp16 via 7-bit digit split of v and hi/lo square
    # splits: d = (vh_j-vh_k)^2 + (vl_j-vl_k)^2, zero iff v_j == v_k.
    # cols 0:L = j-side lhsT rows, cols L:2L = k-side rhs rows.
    quint_d = nc.dram_tensor("quint_d", [BPC, 8, 2 * L], F16, kind="ExternalInput")
    v_i = {x: nc.dram_tensor(f"vi_{x}", [BPC, L], I16, kind="ExternalInput")
           for x in ("s", "d")}
    table = nc.dram_tensor("table", [TROWS, D], F32, kind="ExternalInput")
    feat_t = {"s": nc.dram_tensor("src_feat", [BPC, L, D], F32, kind="ExternalOutput"),
              "d": nc.dram_tensor("dst_feat", [BPC, L, D], F32, kind="ExternalOutput")}
    dbg = {}
    if DEBUG_COUNTS:
        for k in ("ss", "sd", "dd", "ds"):
            dbg[k] = nc.dram_tensor("dbg_" + k, [BPC, 128, NT], F32,
                                    kind="ExternalOutput")

    with tile.TileContext(nc) as tc, ExitStack() as ctx:
        small = ctx.enter_context(tc.tile_pool(name="small", bufs=6))
        bcp = ctx.enter_context(tc.tile_pool(name="bcp", bufs=6))
        pbc = ctx.enter_context(tc.tile_pool(name="pbc", bufs=3, space="PSUM"))
        scr = ctx.enter_context(tc.tile_pool(name="scr", bufs=8))
        feat = ctx.enter_context(tc.tile_pool(name="feat", bufs=8))
        ones = ctx.enter_context(tc.tile_pool(name="ones", bufs=1))
        drsc = ctx.enter_context(tc.tile_pool(name="drsc", bufs=8, space="DRAM"))

        ones_col = ones.tile([128, 1], BF16)
        nc.vector.memset(ones_col[:], 1.0)

        pend = []
        # taper flush groups so the final gathers/stores drain quickly
        flush_after = {2: True, 4: True, 6: True, 7: True}

        for b in range(BPC):
            vv = {}       # [128, NT] f32 (j = t*128 + p), per-partition scalars
            snm = {}      # snap-1 f32
            validv = {}
            vb = {}       # [128, L] int16 broadcast of the v row
            vsn_t = small.tile([128, 2, 2, NT], F32, tag="vsn", name="vsn_t")
            nc.sync.dma_start(out=vsn_t[:], in_=vsn_d[b])
            q5 = small.tile([8, 2 * L], F16, tag="q5", name="q5")
            nc.sync.dma_start(out=q5[:], in_=quint_d[b])
            # flush previous batches' gathers after this batch's loads so the
            # in-order DMA sequencer doesn't head-of-line block the loads
            if pend and flush_after.get(b - 1):
                _flush_gathers(nc, tc, drsc, feat, table, feat_t, pend)
            valid2 = small.tile([128, 2, NT], F32, tag="valid", name="valid2")
            nc.vector.tensor_scalar(
                out=valid2[:], in0=vsn_t[:, 0, :, :], scalar1=8.0, scalar2=None,
                op0=ALU.is_ge)
            sn2 = vsn_t[:, 1, :, :]
            for xi, x in enumerate(("s", "d")):
                vv[x] = vsn_t[:, 0, xi, :]
                vbx = bcp.tile([128, L], I16, tag="vb", name="vb")
                nc.gpsimd.dma_start(out=vbx[:], in_=_replicate_ap(v_i[x][b, :]))
                vb[x] = vbx

            # ---- counting ----
            # cnt_self = [ss | dd], cnt_cross = [sd | ds]
            cnt_self = small.tile([128, 2, NT], F32, tag="cnt_self", name="cnt_self")
            cnt_cross = small.tile([128, 2, NT], F32, tag="cnt_cross", name="cnt_cross")
            cnt = {"ss": cnt_self[:, 0, :], "dd": cnt_self[:, 1, :],
                   "sd": cnt_cross[:, 0, :], "ds": cnt_cross[:, 1, :]}
            for t in range(NT):
                # DVE: src-self
                o = scr.tile([128, L], BF16, tag="scr_v", name="o")
                nc.vector.tensor_scalar(
                    out=o[:], in0=vb["s"][:], scalar1=vv["s"][:, t:t + 1],
                    scalar2=0.0, op0=ALU.is_equal, op1=ALU.add,
                    accum_out=cnt["ss"][:, t:t + 1])
                # DVE: src-cross
                osd = scr.tile([128, L], BF16, tag="scr_sd", name="osd")
                nc.vector.tensor_scalar(
                    out=osd[:], in0=vb["d"][:], scalar1=vv["s"][:, t:t + 1],
                    scalar2=0.0, op0=ALU.is_equal, op1=ALU.add,
                    accum_out=cnt["sd"][:, t:t + 1])
                # DVE: dst-cross
                ods = scr.tile([128, L], BF16, tag="scr_sd", name="ods")
                nc.vector.tensor_scalar(
                    out=ods[:], in0=vb["s"][:], scalar1=vv["d"][:, t:t + 1],
                    scalar2=0.0, op0=ALU.is_equal, op1=ALU.add,
                    accum_out=cnt["ds"][:, t:t + 1])
                # dst-self: first N_ACT_DD tiles on ACT (square + relu(1-x)),
                # the rest on DVE
                if t < N_ACT_DD:
                    # PE computes d = (id_j-id_k)^2 + (sn_j-sn_k)^2; ACT does
                    # relu(1-d) with row-sum accumulation in one pass.
                    d_ps = pbc.tile([128, L], F32, space="PSUM", tag="dps",
                                    name="d_ps")
                    for h in range(2):
                        nc.tensor.matmul(
                            out=d_ps[:, h * 512:(h + 1) * 512],
                            lhsT=q5[:, t * 128:(t + 1) * 128],
                            rhs=q5[:, L + h * 512:L + (h + 1) * 512],
                            start=True, stop=True)
                    o2 = scr.tile([128, L], BF16, tag="scr_a", name="o2")
                    nc.scalar.activation(
                        out=o2[:], in_=d_ps[:], func=ACTF.Relu,
                        bias=1.0, scale=-1.0,
                        accum_out=cnt["dd"][:, t:t + 1])
                else:
                    o3 = scr.tile([128, L], BF16, tag="scr_v", name="o3")
                    nc.vector.tensor_scalar(
                        out=o3[:], in0=vb["d"][:], scalar1=vv["d"][:, t:t + 1],
                        scalar2=0.0, op0=ALU.is_equal, op1=ALU.add,
                        accum_out=cnt["dd"][:, t:t + 1])
            if DEBUG_COUNTS:
                for k in ("ss", "sd", "dd", "ds"):
                    nc.sync.dma_start(out=dbg[k][b], in_=cnt[k][:])

            # ---- table keys:  key = a*CMAX*S + b*S + (sn-1) ----
            a2 = small.tile([128, 2, NT], F32, tag="ka", name="a2")
            nc.vector.tensor_scalar(
                out=a2[:], in0=cnt_self[:], scalar1=float(CMAX - 1),
                scalar2=None, op0=ALU.min)
            nc.vector.tensor_tensor(
                out=a2[:], in0=a2[:], in1=valid2[:], op=ALU.mult)
            b2 = small.tile([128, 2, NT], F32, tag="kb", name="b2")
            nc.vector.tensor_scalar(
                out=b2[:], in0=cnt_cross[:], scalar1=float(CMAX - 1),
                scalar2=None, op0=ALU.min)
            nc.vector.tensor_tensor(
                out=b2[:], in0=b2[:], in1=valid2[:], op=ALU.mult)
            key2 = small.tile([128, 2, NT], F32, tag="key", name="key2")
            nc.vector.scalar_tensor_tensor(
                out=key2[:], in0=a2[:], scalar=float(CMAX * S), in1=sn2,
                op0=ALU.mult, op1=ALU.add)
            nc.vector.scalar_tensor_tensor(
                out=key2[:], in0=b2[:], scalar=float(S), in1=key2[:],
                op0=ALU.mult, op1=ALU.add)
            for xi, x in enumerate(("s", "d")):
                keyi = small.tile([128, NT], I16, tag="keyi", name="keyi")
                nc.vector.tensor_copy(out=keyi[:], in_=key2[:, xi, :])
                pend.append((b, x, keyi))

            if b == BPC - 1:
                _flush_gathers(nc, tc, drsc, feat, table, feat_t, pend)
    nc.compile()
    return nc


def kernel(src_padded_nodes_neighbor_ids, dst_padded_nodes_neighbor_ids,
           src_padded_nodes_snapshots, dst_padded_nodes_snapshots,
           num_snapshots,
           agg_w1, agg_b1, agg_w2, agg_b2, enc_w1, enc_b1, enc_w2, enc_b2):
    tab = build_table(np.asarray(agg_w1), np.asarray(agg_b1),
                      np.asarray(agg_w2), np.asarray(agg_b2),
                      np.asarray(enc_w1), np.asarray(enc_b1),
                      np.asarray(enc_w2), np.asarray(enc_b2))

    if "nc" not in _NC_CACHE:
        _NC_CACHE["nc"] = build_nc()
    nc = _NC_CACHE["nc"]

    ids = {"s": np.asarray(src_padded_nodes_neighbor_ids).astype(np.int64),
           "d": np.asarray(dst_padded_nodes_neighbor_ids).astype(np.int64)}
    sn = {"s": np.asarray(src_padded_nodes_snapshots).astype(np.int64),
          "d": np.asarray(dst_padded_nodes_snapshots).astype(np.int64)}
    v = {x: ids[x] * 8 + (sn[x] - 1) for x in ("s", "d")}

    in_maps = []
    for c in range(NCORES):
        sl = slice(c * BPC, (c + 1) * BPC)
        m = {"table": tab}
        # vsn[b, p, c(v/sn), x(s/d), t]
        vs = np.stack([np.stack([v["s"][sl], v["d"][sl]], axis=1),
                       np.stack([sn["s"][sl] - 1, sn["d"][sl] - 1], axis=1)],
                      axis=1).astype(np.float32)          # [BPC, 2, 2, L]
        vs = vs.reshape(-1, 2, 2, NT, 128).transpose(0, 4, 1, 2, 3)
        m["vsn"] = np.ascontiguousarray(vs)
        vd = v["d"][sl]
        vh = (vd >> 7).astype(np.float64)
        vl = (vd & 127).astype(np.float64)

        def split16(s):
            hi = s.astype(np.float16)
            lo = (s - hi.astype(np.float64)).astype(np.float16)
            return hi, lo

        vh2hi, vh2lo = split16(vh * vh)
        vl2hi, vl2lo = split16(vl * vl)
        k2hi, k2lo = split16(vh * vh + vl * vl)
        one = np.ones_like(vh, dtype=np.float16)
        f16 = np.float16
        qj = np.stack([vh2hi, vh2lo, f16(vh), vl2hi, vl2lo, f16(vl),
                       one, one], axis=1)
        qk = np.stack([one, one, f16(-2.0 * vh), one, one, f16(-2.0 * vl),
                       k2hi, k2lo], axis=1)
        m["quint_d"] = np.ascontiguousarray(
            np.concatenate([qj, qk], axis=2).astype(np.float16))
        for x in ("s", "d"):
            m[f"vi_{x}"] = np.ascontiguousarray(v[x][sl].astype(np.int16))
        in_maps.append(m)
    res = run_bass_kernel_spmd(nc, in_maps, core_ids=list(range(NCORES)),
                               trace=TRACE)
    LAST_RESULTS["res"] = res
    src_feat = np.concatenate([r["src_feat"] for r in res.results], axis=0)
    dst_feat = np.concatenate([r["dst_feat"] for r in res.results], axis=0)
    return (src_feat, dst_feat)



# revision 8
# speedup vs baseline: 1.0705x; 1.0705x over previous
"""Trainium2 Bass kernel for nn_DTFormer (histogram_binning).

Math: for each batch row and each of src/dst lists, count (id,snap)
multiset matches (self and cross), then look the key
key = 256*min(self,31) + 8*min(cross,31) + (snap-1) up in a
host-precomputed table T[8192, 128] (the whole MLP pipeline folded in).

Device work per batch row: the match counting (4 matrices of [L x L]
equality row/col sums) and a row-gather of T, data-parallel over the 64
batch rows across 8 cores.

Counting is split across all engines by (row, list) unit:
 - class-K units: DVE plain is_equal compares in transposed layout
   [k-tile(128) x j(1024)] (4x perf mode, ~450ns) + PE ones-matmul
   column reductions accumulating per-unit count rows in PSUM.
 - class-J units: baseline-style j-tiled compares; most tiles via
   PE fp16-digit distance matmul + ACT relu(1-d) with accum, the rest
   via DVE is_equal with accum (1x, slower, but ACT saturates first).

Validity (id==0 rows) is handled by a host-side sentinel value
(v=16384) on the query side: sentinel never matches any target, so
counts come out 0 and key = snap-1 exactly as the reference requires.

Values are int16 v' reinterpreted as bf16 (bit pattern compare is
exact: no NaN/-0 in range), which is what unlocks the 4x compare.
"""

import sys

for p in ("/opt/trn_rl_repo", "/root/.axon_site/_ro/trn_rl_repo"):
    if p not in sys.path:
        sys.path.insert(0, p)

import numpy as np
import ml_dtypes
from contextlib import ExitStack

import concourse.bass as bass
import concourse.bacc as bacc
import concourse.tile as tile
from concourse import mybir
from concourse.bass_utils import run_bass_kernel_spmd

B, L, S, D = 64, 1024, 8, 128
NCORES = 8
BPC = B // NCORES          # batches per core
NT = L // 128              # j-tiles per row
CMAX = 32
TROWS = CMAX * CMAX * S    # 8192 table rows

K_ROWS = 5                 # rows 0..K_ROWS-1 are class-K, rest class-J
N_ACT_J = 6                # per class-J (row, list): tiles t < N_ACT_J on ACT

F32 = mybir.dt.float32
F16 = mybir.dt.float16
BF16 = mybir.dt.bfloat16
I16 = mybir.dt.int16
ALU = mybir.AluOpType
ACTF = mybir.ActivationFunctionType

_NC_CACHE = {}
TRACE = False
LAST_RESULTS = {}

SENT = 16384               # sentinel v for padding (id == 0) elements

# K units u = 2*r + xi (r < K_ROWS); PSUM partition of unit u:
KGROUPS = [[0, 1, 2, 3], [4, 5, 6, 7], [8, 9]]


def _upart(u):
    return 32 * (u // 4) + (u % 4)


def build_table(agg_w1, agg_b1, agg_w2, agg_b2, enc_w1, enc_b1, enc_w2, enc_b2):
    """T[a*CMAX*S + b*S + s] = output row for (self=a, cross=b, snap=s+1)."""
    a = np.arange(CMAX, dtype=np.float64)
    w1 = agg_w1.astype(np.float64)      # [S, D]
    b1 = agg_b1.astype(np.float64)      # [D]
    ha = np.maximum(a[None, :, None] * w1[:, None, :] + b1, 0.0)  # [S, CMAX, D]
    g = 0.5 * (ha[:, :, None, :] + ha[:, None, :, :])             # [S, A, B, D]
    y = g @ agg_w2.astype(np.float64) + agg_b2.astype(np.float64)  # [S, A, B, 2]
    ew1 = enc_w1.astype(np.float64)[0]   # [D]
    eb1 = enc_b1.astype(np.float64)
    h0 = np.maximum(y[..., 0:1] * ew1 + eb1, 0.0)  # [S, A, B, D]
    h1 = np.maximum(y[..., 1:2] * ew1 + eb1, 0.0)
    out = (h0 + h1) @ enc_w2.astype(np.float64) + 2.0 * enc_b2.astype(np.float64)
    out = np.transpose(out, (1, 2, 0, 3)).reshape(TROWS, D)  # [A,B,S,D] flat
    return np.ascontiguousarray(out.astype(np.float32))


def _replicate_ap(row_ap, parts=128):
    """AP that reads a DRAM row [N] replicated across `parts` partitions."""
    return bass.AP(tensor=row_ap.tensor, offset=row_ap.offset,
                   ap=[[0, parts]] + [list(p) for p in row_ap.ap])


def _flush_gathers(nc, tc, drsc, feat, table, feat_t, pend):
    # round-trip keys through DRAM into the wrapped/replicated int16 idx
    # layout dma_gather expects: idxs[16g + i, w] = key[16w + i] for every g.
    # Keys of several lists are concatenated so the 8 wrap DMAs are shared.
    # k_scr flat order within a block is p*NT + t <-> j = t*128 + p; the
    # gather-slot permutation is undone by the store AP (slot (p, q) holds
    # output row j = 8p + q).
    n = len(pend)
    ctx_hp = tc.high_priority()
    ctx_hp.__enter__()
    k_scr = drsc.tile([1, n * L], I16, tag="kscr", name="k_scr")
    for q, (b_, x_, key_ap) in enumerate(pend):
        dst = k_scr[0, q * L:(q + 1) * L]
        if key_ap.shape[0] == 128:      # class-J [128, NT] tile
            nc.sync.dma_start(
                out=dst.rearrange("(p t) -> p t", t=NT), in_=key_ap)
        else:                           # class-K [1, L] slice, already p-major
            nc.sync.dma_start(out=dst, in_=key_ap)
    idxs_sb = feat.tile([128, n, L // 16], I16, tag="idxs", name="idxs_sb")
    wrap_ap = k_scr[0, :].rearrange("(q i w) -> i q w", i=16, w=L // 16)
    for g in range(8):
        nc.sync.dma_start(out=idxs_sb[16 * g:16 * (g + 1), :, :], in_=wrap_ap)
    for q, (b_, x_, key_ap) in enumerate(pend):
        ft = feat.tile([128, NT, D], F32, tag="ft", name="ft")
        nc.gpsimd.dma_gather(
            out_ap=ft[:], in_ap=table[:],
            idxs_ap=idxs_sb[:, q, :],
            num_idxs=L, num_idxs_reg=L, elem_size=D)
        nc.sync.dma_start(
            out=feat_t[x_][b_, :, :].rearrange("(p q) d -> p q d", q=NT),
            in_=ft[:])
    ctx_hp.__exit__(None, None, None)
    pend.clear()


def build_nc():
    nc = bacc.Bacc("TRN2")
    # vsn[b, p, c(v'/snm1), x(s/d), t]: per-element scalars, j = t*128 + p
    vsn_d = nc.dram_tensor("vsn", [BPC, 128, 2, 2, NT], F32, kind="ExternalInput")
    # vrow[b, x, f]: v' rows (sentinel-ized) in p-major f-order, as bf16 bits
    vrow_d = nc.dram_tensor("vrow", [BPC, 2, L], BF16, kind="ExternalInput")
    # quint[b, c(0..7), x, :]: fp16 digit-split operands; cols 0:L j-side
    # (lhsT), L:2L k-side (rhs)
    quint_d = nc.dram_tensor("quint", [BPC, 8, 2, 2 * L], F16,
                             kind="ExternalInput")
    # kch[p, f]: snap-1 of class-K unit at psum partition p, f-order
    kch_d = nc.dram_tensor("kch", [128, L], F32, kind="ExternalInput")
    # ids4[k, 32*j + c] = 1 iff c == j  (ones-at-column lhsT slices)
    ids4_d = nc.dram_tensor("ids4", [128, 128], BF16, kind="ExternalInput")
    table = nc.dram_tensor("table", [TROWS, D], F32, kind="ExternalInput")
    feat_t = {"s": nc.dram_tensor("src_feat", [BPC, L, D], F32, kind="ExternalOutput"),
              "d": nc.dram_tensor("dst_feat", [BPC, L, D], F32, kind="ExternalOutput")}

    with tile.TileContext(nc) as tc, ExitStack() as ctx:
        cons = ctx.enter_context(tc.tile_pool(name="cons", bufs=1))
        vpool = ctx.enter_context(tc.tile_pool(name="vpool", bufs=1))
        small = ctx.enter_context(tc.tile_pool(name="small", bufs=3))
        scr = ctx.enter_context(tc.tile_pool(name="scr", bufs=8))
        feat = ctx.enter_context(tc.tile_pool(name="feat", bufs=8))
        pbc = ctx.enter_context(tc.tile_pool(name="pbc", bufs=2, space="PSUM"))
        psk = ctx.enter_context(tc.tile_pool(name="psk", bufs=1, space="PSUM"))
        drsc = ctx.enter_context(tc.tile_pool(name="drsc", bufs=8, space="DRAM"))

        ids4 = cons.tile([128, 128], BF16)
        nc.sync.dma_start(out=ids4[:], in_=ids4_d[:, :])
        kch_sb = cons.tile([128, L], F32)
        nc.sync.dma_start(out=kch_sb[:], in_=kch_d[:, :])

        # preload all broadcast value rows (SWDGE; before any gather hits
        # the single software queue)
        vb = {}
        for b in range(BPC):
            for xi, x in enumerate(("s", "d")):
                t_ = vpool.tile([128, L], BF16, tag=f"vb{b}{x}", bufs=1, name="vb")
                nc.gpsimd.dma_start(out=t_[:], in_=_replicate_ap(vrow_d[b, xi, :]))
                vb[(b, x)] = t_

        # class-K count accumulators: psK[c][upart(u), :] = counts of unit u
        psK = {0: psk.tile([128, L], F32, space="PSUM", tag="psks",
                           name="psK_self"),
               1: psk.tile([128, L], F32, space="PSUM", tag="pskc",
                           name="psK_cross")}
        # track first/last MM per (c, 32-block, half) for start/stop flags
        kregion_first = {}

        pend = []

        def load_row(b):
            vsn_t = small.tile([128, 2, 2, NT], F32, tag=f"vsn{b}", bufs=1, name="vsn_t")
            nc.sync.dma_start(out=vsn_t[:], in_=vsn_d[b])
            return vsn_t

        def load_quint(b):
            q5 = small.tile([8, 2, 2 * L], F16, tag="q5", bufs=2, name="q5")
            nc.sync.dma_start(out=q5[:], in_=quint_d[b])
            return q5

        def emit_k_unit(b, xi, x, vsn_t):
            """Class-K: transposed compares + PE column-sum into psK rows."""
            u = 2 * b + xi
            g, j = u // 4, u % 4
            other = "d" if x == "s" else "s"
            lhs = ids4[:, 32 * j:32 * (j + 1)]
            for c, tgt in ((0, x), (1, other)):
                # E_T[k, j'] = [v_tgt[k] == v_x[j']]; scalar side = target
                tgt_xi = 0 if tgt == "s" else 1
                for t in range(NT):
                    e = scr.tile([128, L], BF16, tag="scr_k", name="e")
                    nc.vector.tensor_scalar(
                        out=e[:], in0=vb[(b, x)][:],
                        scalar1=vsn_t[:, 0, tgt_xi, t:t + 1],
                        scalar2=None, op0=ALU.is_equal)
                    for h in range(2):
                        rk = (c, g, h)
                        first = rk not in kregion_first
                        kregion_first[rk] = True
                        last = (j == len(KGROUPS[g]) - 1 or u == 9) and t == NT - 1
                        nc.tensor.matmul(
                            out=psK[c][32 * g:32 * g + 32,
                                       h * 512:(h + 1) * 512],
                            lhsT=lhs, rhs=e[:, h * 512:(h + 1) * 512],
                            start=first, stop=last)

        def assemble_k_group(g):
            us = KGROUPS[g]
            nu = len(us)
            p0 = 32 * g
            c8s = small.tile([128, L], F32, tag="kc8s", bufs=2, name="c8s")
            nc.vector.scalar_tensor_tensor(
                out=c8s[p0:p0 + nu, :], in0=psK[1][p0:p0 + nu, :],
                scalar=8.0, in1=kch_sb[p0:p0 + nu, :],
                op0=ALU.mult, op1=ALU.add)
            keyk = small.tile([128, L], I16, tag="kkey", bufs=2, name="keyk")
            nc.vector.scalar_tensor_tensor(
                out=keyk[p0:p0 + nu, :], in0=psK[0][p0:p0 + nu, :],
                scalar=256.0, in1=c8s[p0:p0 + nu, :],
                op0=ALU.mult, op1=ALU.add)
            for u in us:
                b, xi = u // 2, u % 2
                pend.append((b, "sd"[xi], keyk[p0 + (u % 4):p0 + (u % 4) + 1, :]))

        def emit_j_row(b, vsn_t, q5):
            cnt_self = small.tile([128, 2, NT], F32, tag="cs", name="cnt_self")
            cnt_cross = small.tile([128, 2, NT], F32, tag="cc", name="cnt_cross")
            for xi, x in enumerate(("s", "d")):
                yi = 1 - xi
                for c, (tgt_i, cnt) in enumerate(((xi, cnt_self),
                                                  (yi, cnt_cross))):
                    for t in range(NT):
                        if t < N_ACT_J:
                            d_ps = pbc.tile([128, L], F32, space="PSUM",
                                            tag="dps", name="d_ps")
                            for h in range(2):
                                nc.tensor.matmul(
                                    out=d_ps[:, h * 512:(h + 1) * 512],
                                    lhsT=q5[:, xi, t * 128:(t + 1) * 128],
                                    rhs=q5[:, tgt_i, L + h * 512:L + (h + 1) * 512],
                                    start=True, stop=True)
                            o2 = scr.tile([128, L], BF16, tag="scr_a", name="o2")
                            nc.scalar.activation(
                                out=o2[:], in_=d_ps[:], func=ACTF.Relu,
                                bias=1.0, scale=-1.0,
                                accum_out=cnt[:, xi, t:t + 1])
                        else:
                            tx = "sd"[tgt_i]
                            o3 = scr.tile([128, L], BF16, tag="scr_v", name="o3")
                            nc.vector.tensor_scalar(
                                out=o3[:], in0=vb[(b, tx)][:],
                                scalar1=vsn_t[:, 0, xi, t:t + 1],
                                scalar2=0.0, op0=ALU.is_equal, op1=ALU.add,
                                accum_out=cnt[:, xi, t:t + 1])
            # keys: key = 256*self + 8*cross + (sn-1)
            c8s = small.tile([128, 2, NT], F32, tag="jc8s", name="c8sj")
            nc.vector.scalar_tensor_tensor(
                out=c8s[:], in0=cnt_cross[:], scalar=8.0,
                in1=vsn_t[:, 1, :, :], op0=ALU.mult, op1=ALU.add)
            keyj = small.tile([128, 2, NT], I16, tag="jkey", name="keyj")
            nc.vector.scalar_tensor_tensor(
                out=keyj[:], in0=cnt_self[:], scalar=256.0, in1=c8s[:],
                op0=ALU.mult, op1=ALU.add)
            for xi, x in enumerate(("s", "d")):
                pend.append((b, x, keyj[:, xi, :]))

        # ---- schedule ----
        vsn = {b: load_row(b) for b in range(BPC)}
        # J row 5 first so the gather queue starts early
        q5_5 = load_quint(5)
        emit_j_row(5, vsn[5], q5_5)
        for xi, x in enumerate(("s", "d")):
            emit_k_unit(0, xi, x, vsn[0])
            emit_k_unit(1, xi, x, vsn[1])
        _flush_gathers(nc, tc, drsc, feat, table, feat_t, pend)  # J5 (2)
        for xi, x in enumerate(("s", "d")):
            emit_k_unit(2, xi, x, vsn[2])
            emit_k_unit(3, xi, x, vsn[3])
        assemble_k_group(0)
        _flush_gathers(nc, tc, drsc, feat, table, feat_t, pend)  # K g0 (4)
        q5_6 = load_quint(6)
        emit_j_row(6, vsn[6], q5_6)
        _flush_gathers(nc, tc, drsc, feat, table, feat_t, pend)  # J6 (2)
        for xi, x in enumerate(("s", "d")):
            emit_k_unit(4, xi, x, vsn[4])
        assemble_k_group(1)
        _flush_gathers(nc, tc, drsc, feat, table, feat_t, pend)  # K g1 (4)
        q5_7 = load_quint(7)
        emit_j_row(7, vsn[7], q5_7)
        assemble_k_group(2)
        _flush_gathers(nc, tc, drsc, feat, table, feat_t, pend)  # J7 + K g2 (4)
    nc.compile()
    return nc


def _split16(s):
    hi = s.astype(np.float16)
    lo = (s - hi.astype(np.float64)).astype(np.float16)
    return hi, lo


def _quint(vq, vt):
    """fp16 operands for d[j,k] = (vh_j-vh_k)^2 + (vl_j-vl_k)^2.

    vq: query-side values (j, lhsT cols), vt: target-side values (k)."""
    f16 = np.float16
    one = np.ones_like(vq, dtype=np.float16)

    def side_j(v):
        vh = (v >> 7).astype(np.float64)
        vl = (v & 127).astype(np.float64)
        vh2hi, vh2lo = _split16(vh * vh)
        vl2hi, vl2lo = _split16(vl * vl)
        return np.stack([vh2hi, vh2lo, f16(vh), vl2hi, vl2lo, f16(vl),
                         one, one], axis=0)

    def side_k(v):
        vh = (v >> 7).astype(np.float64)
        vl = (v & 127).astype(np.float64)
        k2hi, k2lo = _split16(vh * vh + vl * vl)
        return np.stack([one, one, f16(-2.0 * vh), one, one, f16(-2.0 * vl),
                         k2hi, k2lo], axis=0)

    return np.concatenate([side_j(vq), side_k(vt)], axis=1).astype(np.float16)


def kernel(src_padded_nodes_neighbor_ids, dst_padded_nodes_neighbor_ids,
           src_padded_nodes_snapshots, dst_padded_nodes_snapshots,
           num_snapshots,
           agg_w1, agg_b1, agg_w2, agg_b2, enc_w1, enc_b1, enc_w2, enc_b2):
    tab = build_table(np.asarray(agg_w1), np.asarray(agg_b1),
                      np.asarray(agg_w2), np.asarray(agg_b2),
                      np.asarray(enc_w1), np.asarray(enc_b1),
                      np.asarray(enc_w2), np.asarray(enc_b2))

    if "nc" not in _NC_CACHE:
        _NC_CACHE["nc"] = build_nc()
    nc = _NC_CACHE["nc"]

    ids = {"s": np.asarray(src_padded_nodes_neighbor_ids).astype(np.int64),
           "d": np.asarray(dst_padded_nodes_neighbor_ids).astype(np.int64)}
    sn = {"s": np.asarray(src_padded_nodes_snapshots).astype(np.int64),
          "d": np.asarray(dst_padded_nodes_snapshots).astype(np.int64)}
    v = {x: ids[x] * 8 + (sn[x] - 1) for x in ("s", "d")}
    vq = {x: np.where(ids[x] == 0, SENT, v[x]) for x in ("s", "d")}

    # count-clamp check: CMAX must bound every multiset multiplicity
    mx = 0
    for x in ("s", "d"):
        for b in range(B):
            mx = max(mx, np.bincount(v[x][b], minlength=SENT + 1).max())
    assert mx < CMAX, f"count overflow: {mx}"

    f = np.arange(L)
    jperm = (f % NT) * 128 + f // NT      # k_scr flat f -> element j

    ids4 = np.zeros((128, 4, 32), dtype=np.float32)
    for j in range(4):
        ids4[:, j, j] = 1.0
    ids4 = ids4.reshape(128, 128).astype(ml_dtypes.bfloat16)

    # scalar channel (vsn c0): class-K rows hold target-side values (real v),
    # class-J rows hold query-side values (sentinel-ized vq).  vrow is the
    # opposite: K query rows sentinel-ized, J target rows real.
    row_is_k = np.arange(BPC) < K_ROWS

    def _reint(a):
        return a.astype(np.int16).view(ml_dtypes.bfloat16).astype(np.float32)

    in_maps = []
    for c in range(NCORES):
        sl = slice(c * BPC, (c + 1) * BPC)
        m = {"table": tab, "ids4": ids4}
        vf = {x: np.where(row_is_k[:, None], _reint(v[x][sl]),
                          _reint(vq[x][sl])) for x in ("s", "d")}
        snm1 = {x: (sn[x][sl] - 1).astype(np.float32) for x in ("s", "d")}
        vs = np.stack([np.stack([vf["s"], vf["d"]], axis=1),
                       np.stack([snm1["s"], snm1["d"]], axis=1)],
                      axis=1)                            # [BPC, 2, 2, L]
        vs = vs.reshape(BPC, 2, 2, NT, 128).transpose(0, 4, 1, 2, 3)
        m["vsn"] = np.ascontiguousarray(vs.astype(np.float32))

        vrow = np.stack(
            [np.where(row_is_k[:, None], vq[x][sl], v[x][sl])[:, jperm]
             for x in ("s", "d")], axis=1).astype(np.int16)  # [BPC, 2, L]
        m["vrow"] = np.ascontiguousarray(vrow).view(ml_dtypes.bfloat16)

        q = np.stack([np.stack([_quint(vq[x][sl][b], v[x][sl][b])
                                for x in ("s", "d")], axis=1)
                      for b in range(BPC)], axis=0)      # [BPC, 8, 2, 2L]
        m["quint"] = np.ascontiguousarray(q.astype(np.float16))

        kch = np.zeros((128, L), dtype=np.float32)
        for u in range(10):
            b, xi = u // 2, u % 2
            x = "sd"[xi]
            kch[_upart(u), :] = snm1[x][b][jperm]
        m["kch"] = kch
        in_maps.append(m)
    res = run_bass_kernel_spmd(nc, in_maps, core_ids=list(range(NCORES)),
                               trace=TRACE)
    LAST_RESULTS["res"] = res
    src_feat = np.concatenate([r["src_feat"] for r in res.results], axis=0)
    dst_feat = np.concatenate([r["dst_feat"] for r in res.results], axis=0)
    return (src_feat, dst_feat)
